# revision 10
# baseline (speedup 1.0000x reference)
# Distributed Trainium2 kernel for the QKV-MLP + causal multi-head attention layer.
#
# Problem (all shapes fixed):
#   x [2, 2048, 1024];  q/k/v = MLP(x) with w1 [1024, 4096] (silu) and w2 [4096, 1024]
#   16 heads x 64 dims, causal softmax attention, output [2, 2048, 1024].
#
# Sharding (8 NeuronCores, single SPMD program; per-core behavior differs only
# through per-core input DATA, never through the instruction stream):
#   - Token/data parallel MLPs: each core computes q/k/v for 512 tokens of one
#     batch (cores 0-3 -> batch 0, 4-7 -> batch 1) with full (replicated) weights.
#   - Core quarter cc owns query tiles [cc, 7-cc, 8+cc, 15-cc] (128 tokens each),
#     which balances causal attention cost (sum of key-tiles = 34 for every cc).
#   - k^T (D-major) and v (token-major, with a ones-column per head for the
#     softmax denominator) are AllGathered within each 4-core batch group.
#   - Attention is query-local: per (head, key-tile) compute S^T = k_tile^T q,
#     exp on ScalarE, causal/validity masking via a host-precomputed per-core
#     mask, then PV accumulation in PSUM (ones-column yields the denominator).
#
# All matmul operands are bf16 (fp32 PSUM accumulation).

import os

import numpy as np
import ml_dtypes

B, S, D, HID, H, DH = 2, 2048, 1024, 4096, 16, 64
P = 128
NCORES = 8
TOK = 512           # tokens per core
NT = S // P         # 16 query/key tiles per batch
VROW = H * (DH + 1)  # 1040: v row with a ones column per head

BF16 = ml_dtypes.bfloat16

_CACHE = {}


def _tiles_for(cc):
    """Global query-tile indices (slot order) owned by quarter cc."""
    return [cc, 7 - cc, 8 + cc, 15 - cc]


def _owner(j):
    """Global tile j -> (owner quarter, slot index)."""
    blk = j // 4
    own = [j, 7 - j, j - 8, 15 - j][blk]
    return own, blk


def _build_program():
    import concourse.bass as bass
    import concourse.mybir as mybir
    import concourse.tile as tile
    from concourse import bacc

    dt = mybir.dt
    AF = mybir.ActivationFunctionType

    # Bacc (not raw Bass): its compile() splits multi-sem waits into event
    # semaphores, which TRN2 engine instructions require (max 1 wait each).
    nc = bacc.Bacc("TRN2", num_devices=NCORES)

    # ---- I/O ----
    xT_d = nc.dram_tensor("xT", [P, 8, TOK], dt.bfloat16, kind="ExternalInput")
    w1_d = {
        m: nc.dram_tensor(f"w1{m}", [HID // P, P, 8, P], dt.bfloat16, kind="ExternalInput")
        for m in "kvq"
    }
    w2kq_d = {
        m: nc.dram_tensor(f"w2{m}", [D // P, P, HID // P, P], dt.bfloat16, kind="ExternalInput")
        for m in "kq"
    }
    w2v_d = nc.dram_tensor("w2v", [HID // P, P, D], dt.bfloat16, kind="ExternalInput")
    b1_d = nc.dram_tensor("b1", [P, 96], dt.float32, kind="ExternalInput")
    b2kq_d = nc.dram_tensor("b2kq", [P, 16], dt.float32, kind="ExternalInput")
    b2v_d = nc.dram_tensor("b2v", [1, D], dt.bfloat16, kind="ExternalInput")
    masks_d = nc.dram_tensor("masks", [P, NT, TOK], dt.bfloat16, kind="ExternalInput")
    o_d = nc.dram_tensor("o", [TOK, D], dt.float32, kind="ExternalOutput")

    with tile.TileContext(nc) as tc:
        with (
            tc.tile_pool(name="constp", bufs=1) as constp,
            tc.tile_pool(name="w1p", bufs=3) as w1p,
            tc.tile_pool(name="w2p", bufs=3) as w2p,
            tc.tile_pool(name="w2vp", bufs=3) as w2vp,
            tc.tile_pool(name="hp", bufs=32) as hp,
            tc.tile_pool(name="qtp", bufs=8) as qtp,
            tc.tile_pool(name="kstp", bufs=3) as kstp,
            tc.tile_pool(name="vstp", bufs=2) as vstp,
            tc.tile_pool(name="kagp", bufs=8) as kagp,
            tc.tile_pool(name="vagp", bufs=16) as vagp,
            tc.tile_pool(name="expp", bufs=5) as expp,
            tc.tile_pool(name="outp", bufs=6) as outp,
            tc.tile_pool(name="psp", bufs=8, space="PSUM") as psp,
            tc.tile_pool(name="dramp", bufs=1, space="DRAM") as dramp,
        ):
            # ---- constants / inputs to SBUF ----
            xt = constp.tile([P, 8, TOK], dt.bfloat16, tag="xt")
            nc.sync.dma_start(xt, xT_d[:, :, :])
            b1_sb = constp.tile([P, 96], dt.float32, tag="b1")
            nc.sync.dma_start(b1_sb, b1_d[:, :])
            b2kq_sb = constp.tile([P, 16], dt.float32, tag="b2kq")
            nc.sync.dma_start(b2kq_sb, b2kq_d[:, :])
            b2v_sb = constp.tile([1, D], dt.bfloat16, tag="b2v")
            nc.sync.dma_start(b2v_sb, b2v_d[:, :])
            masks_sb = constp.tile([P, NT, TOK], dt.bfloat16, tag="masks")
            nc.sync.dma_start(masks_sb, masks_d[:, :, :])
            ones_sb = constp.tile([1, P], dt.bfloat16, tag="ones")
            nc.vector.memset(ones_sb, 1.0)

            # DRAM bounce buffers for the collectives
            k_dram = dramp.tile([D, TOK], dt.bfloat16, tag="k_dram")
            v_dram = dramp.tile([TOK, VROW], dt.bfloat16, tag="v_dram")
            kag_dram = dramp.tile([4 * D, TOK], dt.bfloat16, tag="kag_dram")
            vag_dram = dramp.tile([4 * TOK, VROW], dt.bfloat16, tag="vag_dram")

            groups = [[0, 1, 2, 3], [4, 5, 6, 7]]

            def mlp1(w1d, b1col0):
                """x -> silu(x w1 + b1), transposed layout h^T [HID, TOK] as 32 tiles."""
                hts = []
                for m in range(HID // P):
                    w1t = w1p.tile([P, 8, P], dt.bfloat16, tag="w1t")
                    nc.sync.dma_start(w1t, w1d[m, :, :, :])
                    ps = psp.tile([P, TOK], dt.float32, tag="ps")
                    for kk in range(8):
                        nc.tensor.matmul(
                            ps, lhsT=w1t[:, kk, :], rhs=xt[:, kk, :],
                            start=(kk == 0), stop=(kk == 7),
                        )
                    # silu(t) = t * sigmoid(t), t = ps + b1  (Silu is not in the
                    # interpreter; Sigmoid on ScalarE + fused mul on VectorE)
                    bcol = b1_sb[:, b1col0 + m : b1col0 + m + 1]
                    sg = kstp.tile([P, TOK], dt.float32, tag="sg")
                    nc.scalar.activation(sg, ps, AF.Sigmoid, bias=bcol)
                    ht = hp.tile([P, TOK], dt.bfloat16, tag="ht")
                    nc.vector.scalar_tensor_tensor(
                        ht, ps, bcol, sg,
                        op0=mybir.AluOpType.add, op1=mybir.AluOpType.mult,
                    )
                    hts.append(ht)
                return hts

            def mlp2_kq(w2d, b2col0, hts, dest):
                """h^T -> (silu_h w2 + b2)^T, D-major [D, TOK]; dest 'k' -> k_dram, 'q' -> SBUF."""
                q_tiles = []
                for md in range(D // P):
                    ps = psp.tile([P, TOK], dt.float32, tag="ps")
                    for kc in range(4):
                        w2t = w2p.tile([P, 8, P], dt.bfloat16, tag="w2t")
                        nc.sync.dma_start(w2t, w2d[md, :, 8 * kc : 8 * kc + 8, :])
                        for i in range(8):
                            kk = 8 * kc + i
                            nc.tensor.matmul(
                                ps, lhsT=w2t[:, i, :], rhs=hts[kk],
                                start=(kk == 0), stop=(kk == 31),
                            )
                    if dest == "k":
                        kst = kstp.tile([P, TOK], dt.bfloat16, tag="kst")
                        nc.scalar.activation(
                            kst, ps, AF.Identity, bias=b2kq_sb[:, b2col0 + md : b2col0 + md + 1]
                        )
                        nc.sync.dma_start(k_dram[P * md : P * (md + 1), :], kst)
                    else:
                        qt = qtp.tile([P, TOK], dt.bfloat16, tag="qt")
                        nc.scalar.activation(
                            qt, ps, AF.Identity, bias=b2kq_sb[:, b2col0 + md : b2col0 + md + 1]
                        )
                        q_tiles.append(qt)
                return q_tiles

            def mlp2_v(hts):
                """h^T -> v token-major [TOK, VROW] with ones cols, written to v_dram."""
                vps = [psp.tile([P, TOK], dt.float32, tag="ps", name=f"vps{i}") for i in range(8)]
                for kk in range(HID // P):
                    w2vt = w2vp.tile([P, D], dt.bfloat16, tag="w2vt")
                    nc.sync.dma_start(w2vt, w2v_d[kk, :, :])
                    for mt in range(4):
                        for n2 in range(2):
                            nc.tensor.matmul(
                                vps[mt * 2 + n2],
                                lhsT=hts[kk][:, P * mt : P * (mt + 1)],
                                rhs=w2vt[:, 512 * n2 : 512 * (n2 + 1)],
                                start=(kk == 0), stop=False,
                            )
                for idx in range(8):
                    n2 = idx % 2
                    nc.tensor.matmul(
                        vps[idx], lhsT=ones_sb[0:1, 0:P],
                        rhs=b2v_sb[0:1, 512 * n2 : 512 * (n2 + 1)],
                        start=False, stop=True,
                    )
                for mt in range(4):
                    vst = vstp.tile([P, VROW], dt.bfloat16, tag="vst")
                    for n2 in range(2):
                        for h8 in range(8):
                            h = 8 * n2 + h8
                            nc.scalar.activation(
                                vst[:, 65 * h : 65 * h + 64],
                                vps[mt * 2 + n2][:, 64 * h8 : 64 * h8 + 64],
                                AF.Copy,
                            )
                    nc.vector.memset(
                        vst.rearrange("p (h c) -> p h c", c=65)[:, :, 64:65], 1.0
                    )
                    nc.sync.dma_start(v_dram[P * mt : P * (mt + 1), :], vst)

            # ---- MLPs: k, v, q (k/v first so their AllGathers overlap q's MLP) ----
            hts = mlp1(w1_d["k"], 0)
            mlp2_kq(w2kq_d["k"], 0, hts, "k")
            nc.gpsimd.collective_compute(
                "AllGather", mybir.AluOpType.bypass, replica_groups=groups,
                ins=[k_dram.opt()], outs=[kag_dram.opt()],
            )

            hts = mlp1(w1_d["v"], 32)
            mlp2_v(hts)
            nc.gpsimd.collective_compute(
                "AllGather", mybir.AluOpType.bypass, replica_groups=groups,
                ins=[v_dram.opt()], outs=[vag_dram.opt()],
            )

            hts = mlp1(w1_d["q"], 64)
            q_tiles = mlp2_kq(w2kq_d["q"], 8, hts, "q")

            # ---- attention ----
            # Load gathered v (whole batch) into SBUF: 16 tiles [128, 1040].
            vag_sb = []
            for vt in range(16):
                vgt = vagp.tile([P, VROW], dt.bfloat16, tag="vgt")
                nc.sync.dma_start(vgt, vag_dram[P * vt : P * (vt + 1), :])
                vag_sb.append(vgt)

            for pair in range(H // 2):
                # k rows for heads (2*pair, 2*pair+1) from each of the 4 owners
                kag_sb = []
                for own in range(4):
                    kgt = kagp.tile([P, TOK], dt.bfloat16, tag="kgt")
                    rt = 8 * own + pair
                    nc.sync.dma_start(kgt, kag_dram[P * rt : P * (rt + 1), :])
                    kag_sb.append(kgt)
                for hh in range(2):
                    h = 2 * pair + hh
                    po = 64 * hh
                    pv = [psp.tile([P, TOK], dt.float32, tag="ps", name=f"pv{i}") for i in range(4)]
                    for j in range(NT):
                        own, blk = _owner(j)
                        c0 = P * blk  # first valid local query column
                        ps_s = psp.tile([P, TOK], dt.float32, tag="ps")
                        nc.tensor.matmul(
                            ps_s[:, c0:TOK],
                            lhsT=kag_sb[own][po : po + 64, P * blk : P * (blk + 1)],
                            rhs=q_tiles[h // 2][po : po + 64, c0:TOK],
                            start=True, stop=True,
                        )
                        ex = expp.tile([P, TOK], dt.bfloat16, tag="ex")
                        nc.scalar.activation(
                            ex[:, c0:TOK], ps_s[:, c0:TOK], AF.Exp, scale=0.125
                        )
                        nc.vector.tensor_mul(
                            ex[:, c0:TOK], ex[:, c0:TOK], masks_sb[:, j, c0:TOK]
                        )
                        vt = 4 * own + blk
                        for p in range(blk, 4):
                            nc.tensor.matmul(
                                pv[p][:, 0:65],
                                lhsT=ex[:, P * p : P * (p + 1)],
                                rhs=vag_sb[vt][:, 65 * h : 65 * h + 65],
                                start=(j == 0), stop=(j == 4 * p + 3),
                            )
                    for p in range(4):
                        rec = outp.tile([P, 1], dt.float32, tag="rec")
                        nc.vector.reciprocal(rec, pv[p][:, 64:65])
                        ot = outp.tile([P, 64], dt.float32, tag="ot")
                        nc.scalar.activation(ot, pv[p][:, 0:64], AF.Copy, scale=rec)
                        nc.sync.dma_start(
                            o_d[P * p : P * (p + 1), 64 * h : 64 * h + 64], ot
                        )
    nc.compile()
    return nc


def _host_inputs(inputs):
    """Build the 8 per-core input maps from the full-problem inputs."""
    x = np.ascontiguousarray(inputs["x"]).astype(np.float32)

    def pack_w1(w1):
        return np.ascontiguousarray(
            w1.astype(BF16).reshape(8, P, HID // P, P).transpose(2, 1, 0, 3)
        )

    def pack_w2(w2):
        return np.ascontiguousarray(
            w2.astype(BF16).reshape(HID // P, P, D // P, P).transpose(2, 1, 0, 3)
        )

    shared = {
        "w1k": pack_w1(np.asarray(inputs["k_w1"])),
        "w1v": pack_w1(np.asarray(inputs["v_w1"])),
        "w1q": pack_w1(np.asarray(inputs["q_w1"])),
        "w2k": pack_w2(np.asarray(inputs["k_w2"])),
        "w2q": pack_w2(np.asarray(inputs["q_w2"])),
        "w2v": np.ascontiguousarray(
            np.asarray(inputs["v_w2"]).astype(BF16).reshape(HID // P, P, D)
        ),
        "b1": np.ascontiguousarray(
            np.concatenate(
                [np.asarray(inputs[m + "_b1"]).astype(np.float32).reshape(HID // P, P).T
                 for m in "kvq"], axis=1)
        ),
        "b2kq": np.ascontiguousarray(
            np.concatenate(
                [np.asarray(inputs[m + "_b2"]).astype(np.float32).reshape(D // P, P).T
                 for m in "kq"], axis=1)
        ),
        "b2v": np.ascontiguousarray(np.asarray(inputs["v_b2"]).astype(BF16).reshape(1, D)),
    }

    in_maps = []
    for c in range(NCORES):
        b, cc = divmod(c, 4)
        tiles = _tiles_for(cc)
        tok = np.concatenate([np.arange(P * t, P * (t + 1)) for t in tiles])
        xT = x[b].T[:, tok]  # [D, TOK]
        xT_packed = np.ascontiguousarray(
            xT.astype(BF16).reshape(8, P, TOK).transpose(1, 0, 2)
        )
        # mask[p_k, j, f_q] = 1 if key (128 j + p_k) <= query(local col f_q) else 0
        pk = np.arange(P)
        jj = np.arange(NT)
        fq = np.arange(TOK)
        qglob = np.array([P * tiles[f // P] + f % P for f in fq])  # [TOK]
        keyglob = P * jj[None, :, None] + pk[:, None, None]  # [P, NT, 1]
        mask = (keyglob <= qglob[None, None, :]).astype(BF16)
        in_maps.append({**shared, "xT": xT_packed, "masks": np.ascontiguousarray(mask)})
    return in_maps


LAST_RESULT = None


def kernel(**inputs):
    global LAST_RESULT
    key = "prog"
    if key not in _CACHE:
        _CACHE[key] = _build_program()
    nc = _CACHE[key]

    from concourse.bass_utils import run_bass_kernel_spmd

    in_maps = _host_inputs(inputs)
    res = run_bass_kernel_spmd(nc, in_maps, core_ids=list(range(NCORES)))
    LAST_RESULT = res

    full = np.zeros((B, S, D), np.float32)
    for c in range(NCORES):
        b, cc = divmod(c, 4)
        o_c = res.results[c]["o"]
        for p, t in enumerate(_tiles_for(cc)):
            full[b, P * t : P * (t + 1), :] = o_c[P * p : P * (p + 1), :]
    return full.astype(inputs["x"].dtype if hasattr(inputs["x"], "dtype") else np.float32)


# revision 15
# speedup vs baseline: 1.0328x; 1.0328x over previous
# Distributed Trainium2 kernel for the QKV-MLP + causal multi-head attention layer.
#
# Problem (all shapes fixed):
#   x [2, 2048, 1024];  q/k/v = MLP(x) with w1 [1024, 4096] (silu) and w2 [4096, 1024]
#   16 heads x 64 dims, causal softmax attention, output [2, 2048, 1024].
#
# Sharding (8 NeuronCores, single SPMD program; per-core behavior differs only
# through per-core input DATA, never through the instruction stream):
#   - Token/data parallel MLPs: each core computes q/k/v for 512 tokens of one
#     batch (cores 0-3 -> batch 0, 4-7 -> batch 1) with full (replicated) weights.
#   - Core quarter cc owns query tiles [cc, 7-cc, 8+cc, 15-cc] (128 tokens each),
#     which balances causal attention cost (sum of key-tiles = 34 for every cc).
#   - k^T (D-major) and v (token-major, with a ones-column per head for the
#     softmax denominator) are AllGathered within each 4-core batch group.
#   - Attention is query-local: per (head, key-tile) compute S^T = k_tile^T q,
#     exp on ScalarE, causal/validity masking via a host-precomputed per-core
#     mask, then PV accumulation in PSUM (ones-column yields the denominator).
#
# All matmul operands are bf16 (fp32 PSUM accumulation).

import os

import numpy as np
import ml_dtypes

B, S, D, HID, H, DH = 2, 2048, 1024, 4096, 16, 64
P = 128
NCORES = 8
TOK = 512           # tokens per core
NT = S // P         # 16 query/key tiles per batch
VROW = H * (DH + 1)  # 1040: v row with a ones column per head

BF16 = ml_dtypes.bfloat16

_CACHE = {}


def _tiles_for(cc):
    """Global query-tile indices (slot order) owned by quarter cc."""
    return [cc, 7 - cc, 8 + cc, 15 - cc]


def _owner(j):
    """Global tile j -> (owner quarter, slot index)."""
    blk = j // 4
    own = [j, 7 - j, j - 8, 15 - j][blk]
    return own, blk


def _build_program():
    import concourse.bass as bass
    import concourse.mybir as mybir
    import concourse.tile as tile
    from concourse import bacc

    dt = mybir.dt
    AF = mybir.ActivationFunctionType

    # Bacc (not raw Bass): its compile() splits multi-sem waits into event
    # semaphores, which TRN2 engine instructions require (max 1 wait each).
    nc = bacc.Bacc("TRN2", num_devices=NCORES)

    # ---- I/O ----
    xT_d = nc.dram_tensor("xT", [P, 8, TOK], dt.bfloat16, kind="ExternalInput")
    w1_d = {
        m: nc.dram_tensor(f"w1{m}", [HID // P, P, 8, P], dt.bfloat16, kind="ExternalInput")
        for m in "kvq"
    }
    w2kq_d = {
        m: nc.dram_tensor(f"w2{m}", [D // P, P, HID // P, P], dt.bfloat16, kind="ExternalInput")
        for m in "kq"
    }
    w2v_d = nc.dram_tensor("w2v", [HID // P, P, D], dt.bfloat16, kind="ExternalInput")
    b1_d = nc.dram_tensor("b1", [P, 96], dt.float32, kind="ExternalInput")
    b2kq_d = nc.dram_tensor("b2kq", [P, 16], dt.float32, kind="ExternalInput")
    b2v_d = nc.dram_tensor("b2v", [1, D], dt.bfloat16, kind="ExternalInput")
    masks_d = nc.dram_tensor("masks", [P, NT, TOK], dt.bfloat16, kind="ExternalInput")
    o_d = nc.dram_tensor("o", [TOK, D], dt.float32, kind="ExternalOutput")

    with tile.TileContext(nc) as tc:
        with (
            tc.tile_pool(name="constp", bufs=1) as constp,
            tc.tile_pool(name="w1p", bufs=3) as w1p,
            tc.tile_pool(name="w2p", bufs=3) as w2p,
            tc.tile_pool(name="w2vp", bufs=3) as w2vp,
            tc.tile_pool(name="hp", bufs=64) as hp,
            tc.tile_pool(name="qtp", bufs=8) as qtp,
            tc.tile_pool(name="kstp", bufs=3) as kstp,
            tc.tile_pool(name="vstp", bufs=2) as vstp,
            tc.tile_pool(name="kagp", bufs=8) as kagp,
            tc.tile_pool(name="vagp", bufs=16) as vagp,
            tc.tile_pool(name="expp", bufs=20) as expp,
            tc.tile_pool(name="outp", bufs=6) as outp,
            tc.tile_pool(name="psp", bufs=8, space="PSUM") as psp,
            tc.tile_pool(name="dramp", bufs=1, space="DRAM") as dramp,
        ):
            # ---- constants / inputs to SBUF ----
            xt = constp.tile([P, 8, TOK], dt.bfloat16, tag="xt")
            nc.sync.dma_start(xt, xT_d[:, :, :])
            b1_sb = constp.tile([P, 96], dt.float32, tag="b1")
            nc.sync.dma_start(b1_sb, b1_d[:, :])
            b2kq_sb = constp.tile([P, 16], dt.float32, tag="b2kq")
            nc.sync.dma_start(b2kq_sb, b2kq_d[:, :])
            b2v_sb = constp.tile([1, D], dt.bfloat16, tag="b2v")
            nc.sync.dma_start(b2v_sb, b2v_d[:, :])
            masks_sb = constp.tile([P, NT, TOK], dt.bfloat16, tag="masks")
            nc.sync.dma_start(masks_sb, masks_d[:, :, :])
            ones_sb = constp.tile([1, P], dt.bfloat16, tag="ones")
            nc.vector.memset(ones_sb, 1.0)

            # DRAM bounce buffers for the collectives
            k_dram = dramp.tile([D, TOK], dt.bfloat16, tag="k_dram")
            v_dram = dramp.tile([TOK, VROW], dt.bfloat16, tag="v_dram")
            kag_dram = dramp.tile([4 * D, TOK], dt.bfloat16, tag="kag_dram")
            vag_dram = dramp.tile([4 * TOK, VROW], dt.bfloat16, tag="vag_dram")

            groups = [[0, 1, 2, 3], [4, 5, 6, 7]]

            def mlp1(w1d, b1col0):
                """x -> silu(x w1 + b1), transposed layout h^T [HID, TOK] as 32 tiles."""
                hts = []
                for m in range(HID // P):
                    w1t = w1p.tile([P, 8, P], dt.bfloat16, tag="w1t")
                    nc.sync.dma_start(w1t, w1d[m, :, :, :])
                    ps = psp.tile([P, TOK], dt.float32, tag="ps")
                    for kk in range(8):
                        nc.tensor.matmul(
                            ps, lhsT=w1t[:, kk, :], rhs=xt[:, kk, :],
                            start=(kk == 0), stop=(kk == 7),
                        )
                    # silu(t) = t * sigmoid(t), t = ps + b1  (Silu is not in the
                    # interpreter; Sigmoid on ScalarE + fused mul on VectorE)
                    bcol = b1_sb[:, b1col0 + m : b1col0 + m + 1]
                    sg = kstp.tile([P, TOK], dt.float32, tag="sg")
                    nc.scalar.activation(sg, ps, AF.Sigmoid, bias=bcol)
                    ht = hp.tile([P, TOK], dt.bfloat16, tag="ht")
                    nc.vector.scalar_tensor_tensor(
                        ht, ps, bcol, sg,
                        op0=mybir.AluOpType.add, op1=mybir.AluOpType.mult,
                    )
                    hts.append(ht)
                return hts

            def mlp2_kq(w2d, b2col0, hts, dest):
                """h^T -> (silu_h w2 + b2)^T, D-major [D, TOK]; dest 'k' -> k_dram, 'q' -> SBUF."""
                q_tiles = []
                for md in range(D // P):
                    ps = psp.tile([P, TOK], dt.float32, tag="ps")
                    for kc in range(4):
                        w2t = w2p.tile([P, 8, P], dt.bfloat16, tag="w2t")
                        nc.sync.dma_start(w2t, w2d[md, :, 8 * kc : 8 * kc + 8, :])
                        for i in range(8):
                            kk = 8 * kc + i
                            nc.tensor.matmul(
                                ps, lhsT=w2t[:, i, :], rhs=hts[kk],
                                start=(kk == 0), stop=(kk == 31),
                            )
                    if dest == "k":
                        kst = kstp.tile([P, TOK], dt.bfloat16, tag="kst")
                        nc.scalar.activation(
                            kst, ps, AF.Identity, bias=b2kq_sb[:, b2col0 + md : b2col0 + md + 1]
                        )
                        nc.sync.dma_start(k_dram[P * md : P * (md + 1), :], kst)
                    else:
                        qt = qtp.tile([P, TOK], dt.bfloat16, tag="qt")
                        nc.scalar.activation(
                            qt, ps, AF.Identity, bias=b2kq_sb[:, b2col0 + md : b2col0 + md + 1]
                        )
                        q_tiles.append(qt)
                return q_tiles

            def mlp2_v(hts):
                """h^T -> v token-major [TOK, VROW] with ones cols, written to v_dram."""
                vps = [psp.tile([P, TOK], dt.float32, tag="ps", name=f"vps{i}") for i in range(8)]
                for kk in range(HID // P):
                    w2vt = w2vp.tile([P, D], dt.bfloat16, tag="w2vt")
                    nc.sync.dma_start(w2vt, w2v_d[kk, :, :])
                    for mt in range(4):
                        for n2 in range(2):
                            nc.tensor.matmul(
                                vps[mt * 2 + n2],
                                lhsT=hts[kk][:, P * mt : P * (mt + 1)],
                                rhs=w2vt[:, 512 * n2 : 512 * (n2 + 1)],
                                start=(kk == 0), stop=False,
                            )
                for idx in range(8):
                    n2 = idx % 2
                    nc.tensor.matmul(
                        vps[idx], lhsT=ones_sb[0:1, 0:P],
                        rhs=b2v_sb[0:1, 512 * n2 : 512 * (n2 + 1)],
                        start=False, stop=True,
                    )
                for mt in range(4):
                    vst = vstp.tile([P, VROW], dt.bfloat16, tag="vst")
                    for n2 in range(2):
                        for h8 in range(8):
                            h = 8 * n2 + h8
                            nc.scalar.activation(
                                vst[:, 65 * h : 65 * h + 64],
                                vps[mt * 2 + n2][:, 64 * h8 : 64 * h8 + 64],
                                AF.Copy,
                            )
                    nc.vector.memset(
                        vst.rearrange("p (h c) -> p h c", c=65)[:, :, 64:65], 1.0
                    )
                    nc.sync.dma_start(v_dram[P * mt : P * (mt + 1), :], vst)

            # ---- MLPs: v, k, q (v first so its AllGather finishes earliest) ----
            hts = mlp1(w1_d["v"], 32)
            mlp2_v(hts)
            nc.gpsimd.collective_compute(
                "AllGather", mybir.AluOpType.bypass, replica_groups=groups,
                ins=[v_dram.opt()], outs=[vag_dram.opt()],
            )

            hts = mlp1(w1_d["k"], 0)
            mlp2_kq(w2kq_d["k"], 0, hts, "k")
            nc.gpsimd.collective_compute(
                "AllGather", mybir.AluOpType.bypass, replica_groups=groups,
                ins=[k_dram.opt()], outs=[kag_dram.opt()],
            )

            hts = mlp1(w1_d["q"], 64)
            q_tiles = mlp2_kq(w2kq_d["q"], 8, hts, "q")

            # ---- attention ----
            # Load gathered v (whole batch) into SBUF: 16 tiles [128, 1040].
            vag_sb = []
            for vt in range(16):
                vgt = vagp.tile([P, VROW], dt.bfloat16, tag="vgt")
                nc.sync.dma_start(vgt, vag_dram[P * vt : P * (vt + 1), :])
                vag_sb.append(vgt)

            for pair in range(H // 2):
                # k rows for heads (2*pair, 2*pair+1) from each of the 4 owners
                kag_sb = []
                for own in range(4):
                    kgt = kagp.tile([P, TOK], dt.bfloat16, tag="kgt")
                    rt = 8 * own + pair
                    nc.sync.dma_start(kgt, kag_dram[P * rt : P * (rt + 1), :])
                    kag_sb.append(kgt)
                # Per head, two phases: (1) S^T + exp + mask for all key tiles
                # (kept in SBUF), (2) PV accumulation per query slot — one
                # PSUM accumulation group at a time (groups are bank-granular).
                for hh in range(2):
                    h = 2 * pair + hh
                    po = 64 * hh
                    exs = []
                    for j in range(NT):
                        own, blk = _owner(j)
                        c0 = P * blk  # first valid local query column
                        ps_s = psp.tile([P, TOK], dt.float32, tag="ps")
                        nc.tensor.matmul(
                            ps_s[:, c0:TOK],
                            lhsT=kag_sb[own][po : po + 64, P * blk : P * (blk + 1)],
                            rhs=q_tiles[h // 2][po : po + 64, c0:TOK],
                            start=True, stop=True,
                        )
                        ex = expp.tile([P, TOK], dt.bfloat16, tag="ex")
                        nc.scalar.activation(
                            ex[:, c0:TOK], ps_s[:, c0:TOK], AF.Exp, scale=0.125
                        )
                        nc.vector.tensor_mul(
                            ex[:, c0:TOK], ex[:, c0:TOK], masks_sb[:, j, c0:TOK]
                        )
                        exs.append(ex)
                    for p in range(4):
                        pvt = psp.tile([P, TOK], dt.float32, tag="ps", name=f"pvt{p}")
                        for j in range(4 * p + 4):
                            own, blk = _owner(j)
                            nc.tensor.matmul(
                                pvt[:, 0:65],
                                lhsT=exs[j][:, P * p : P * (p + 1)],
                                rhs=vag_sb[4 * own + blk][:, 65 * h : 65 * h + 65],
                                start=(j == 0), stop=(j == 4 * p + 3),
                            )
                        rec = outp.tile([P, 1], dt.float32, tag="rec")
                        nc.vector.reciprocal(rec, pvt[:, 64:65])
                        ot = outp.tile([P, 64], dt.float32, tag="ot")
                        nc.scalar.activation(ot, pvt[:, 0:64], AF.Copy, scale=rec)
                        nc.sync.dma_start(
                            o_d[P * p : P * (p + 1), 64 * h : 64 * h + 64], ot
                        )
    nc.compile()
    return nc


def _host_inputs(inputs):
    """Build the 8 per-core input maps from the full-problem inputs."""
    x = np.ascontiguousarray(inputs["x"]).astype(np.float32)

    def pack_w1(w1):
        return np.ascontiguousarray(
            w1.astype(BF16).reshape(8, P, HID // P, P).transpose(2, 1, 0, 3)
        )

    def pack_w2(w2):
        return np.ascontiguousarray(
            w2.astype(BF16).reshape(HID // P, P, D // P, P).transpose(2, 1, 0, 3)
        )

    shared = {
        "w1k": pack_w1(np.asarray(inputs["k_w1"])),
        "w1v": pack_w1(np.asarray(inputs["v_w1"])),
        "w1q": pack_w1(np.asarray(inputs["q_w1"])),
        "w2k": pack_w2(np.asarray(inputs["k_w2"])),
        "w2q": pack_w2(np.asarray(inputs["q_w2"])),
        "w2v": np.ascontiguousarray(
            np.asarray(inputs["v_w2"]).astype(BF16).reshape(HID // P, P, D)
        ),
        "b1": np.ascontiguousarray(
            np.concatenate(
                [np.asarray(inputs[m + "_b1"]).astype(np.float32).reshape(HID // P, P).T
                 for m in "kvq"], axis=1)
        ),
        "b2kq": np.ascontiguousarray(
            np.concatenate(
                [np.asarray(inputs[m + "_b2"]).astype(np.float32).reshape(D // P, P).T
                 for m in "kq"], axis=1)
        ),
        "b2v": np.ascontiguousarray(np.asarray(inputs["v_b2"]).astype(BF16).reshape(1, D)),
    }

    in_maps = []
    for c in range(NCORES):
        b, cc = divmod(c, 4)
        tiles = _tiles_for(cc)
        tok = np.concatenate([np.arange(P * t, P * (t + 1)) for t in tiles])
        xT = x[b].T[:, tok]  # [D, TOK]
        xT_packed = np.ascontiguousarray(
            xT.astype(BF16).reshape(8, P, TOK).transpose(1, 0, 2)
        )
        # mask[p_k, j, f_q] = 1 if key (128 j + p_k) <= query(local col f_q) else 0
        pk = np.arange(P)
        jj = np.arange(NT)
        fq = np.arange(TOK)
        qglob = np.array([P * tiles[f // P] + f % P for f in fq])  # [TOK]
        keyglob = P * jj[None, :, None] + pk[:, None, None]  # [P, NT, 1]
        mask = (keyglob <= qglob[None, None, :]).astype(BF16)
        in_maps.append({**shared, "xT": xT_packed, "masks": np.ascontiguousarray(mask)})
    return in_maps


LAST_RESULT = None


def kernel(**inputs):
    global LAST_RESULT
    key = "prog"
    if key not in _CACHE:
        _CACHE[key] = _build_program()
    nc = _CACHE[key]

    from concourse.bass_utils import run_bass_kernel_spmd

    in_maps = _host_inputs(inputs)
    res = run_bass_kernel_spmd(nc, in_maps, core_ids=list(range(NCORES)))
    LAST_RESULT = res

    full = np.zeros((B, S, D), np.float32)
    for c in range(NCORES):
        b, cc = divmod(c, 4)
        o_c = res.results[c]["o"]
        for p, t in enumerate(_tiles_for(cc)):
            full[b, P * t : P * (t + 1), :] = o_c[P * p : P * (p + 1), :]
    return full.astype(inputs["x"].dtype if hasattr(inputs["x"], "dtype") else np.float32)


# revision 21
# speedup vs baseline: 1.1393x; 1.1031x over previous
# Distributed Trainium2 kernel for the QKV-MLP + causal multi-head attention layer.
#
# Problem (all shapes fixed):
#   x [2, 2048, 1024];  q/k/v = MLP(x) with w1 [1024, 4096] (silu) and w2 [4096, 1024]
#   16 heads x 64 dims, causal softmax attention, output [2, 2048, 1024].
#
# Sharding (8 NeuronCores, single SPMD program; per-core behavior differs only
# through per-core input DATA, never through the instruction stream):
#   - Token/data parallel MLPs: each core computes q/k/v for 512 tokens of one
#     batch (cores 0-3 -> batch 0, 4-7 -> batch 1) with full (replicated) weights.
#   - Core quarter cc owns query tiles [cc, 7-cc, 8+cc, 15-cc] (128 tokens each),
#     which balances causal attention cost (sum of key-tiles = 34 for every cc).
#   - k^T (D-major) and v (token-major, with a ones-column per head for the
#     softmax denominator) are AllGathered within each 4-core batch group.
#   - Attention is query-local: per (head, key-tile) compute S^T = k_tile^T q,
#     exp on ScalarE, causal/validity masking via a host-precomputed per-core
#     mask, then PV accumulation in PSUM (ones-column yields the denominator).
#
# All matmul operands are bf16 (fp32 PSUM accumulation).

import os

import numpy as np
import ml_dtypes

B, S, D, HID, H, DH = 2, 2048, 1024, 4096, 16, 64
P = 128
NCORES = 8
TOK = 512           # tokens per core
NT = S // P         # 16 query/key tiles per batch
VROW = H * (DH + 1)  # 1040: v row with a ones column per head

BF16 = ml_dtypes.bfloat16

_CACHE = {}


def _tiles_for(cc):
    """Global query-tile indices (slot order) owned by quarter cc."""
    return [cc, 7 - cc, 8 + cc, 15 - cc]


def _owner(j):
    """Global tile j -> (owner quarter, slot index)."""
    blk = j // 4
    own = [j, 7 - j, j - 8, 15 - j][blk]
    return own, blk


def _build_program():
    import concourse.bass as bass
    import concourse.mybir as mybir
    import concourse.tile as tile
    from concourse import bacc

    dt = mybir.dt
    AF = mybir.ActivationFunctionType

    # Bacc (not raw Bass): its compile() splits multi-sem waits into event
    # semaphores, which TRN2 engine instructions require (max 1 wait each).
    nc = bacc.Bacc("TRN2", num_devices=NCORES)

    # ---- I/O ----
    xT_d = nc.dram_tensor("xT", [P, 8, TOK], dt.bfloat16, kind="ExternalInput")
    w1_d = {
        m: nc.dram_tensor(f"w1{m}", [HID // P, P, 8, P], dt.bfloat16, kind="ExternalInput")
        for m in "kvq"
    }
    w2kq_d = {
        m: nc.dram_tensor(f"w2{m}", [D // P, P, HID // P, P], dt.bfloat16, kind="ExternalInput")
        for m in "kq"
    }
    w2v_d = nc.dram_tensor("w2v", [HID // P, P, D], dt.bfloat16, kind="ExternalInput")
    b1_d = nc.dram_tensor("b1", [P, 96], dt.float32, kind="ExternalInput")
    b2kq_d = nc.dram_tensor("b2kq", [P, 16], dt.float32, kind="ExternalInput")
    b2v_d = nc.dram_tensor("b2v", [1, D], dt.bfloat16, kind="ExternalInput")
    masks_d = nc.dram_tensor("masks", [P, NT, TOK], dt.bfloat16, kind="ExternalInput")
    o_d = nc.dram_tensor("o", [TOK, D], dt.float32, kind="ExternalOutput")

    with tile.TileContext(nc) as tc:
        with (
            tc.tile_pool(name="constp", bufs=1) as constp,
            tc.tile_pool(name="w1p", bufs=6) as w1p,
            tc.tile_pool(name="w2p", bufs=4) as w2p,
            tc.tile_pool(name="w2vp", bufs=4) as w2vp,
            tc.tile_pool(name="hp", bufs=64) as hp,
            tc.tile_pool(name="qtp", bufs=8) as qtp,
            tc.tile_pool(name="kstp", bufs=3) as kstp,
            tc.tile_pool(name="vstp", bufs=2) as vstp,
            tc.tile_pool(name="kagp", bufs=8) as kagp,
            tc.tile_pool(name="vagp", bufs=16) as vagp,
            tc.tile_pool(name="expp", bufs=20) as expp,
            tc.tile_pool(name="outp", bufs=6) as outp,
            tc.tile_pool(name="psp", bufs=8, space="PSUM") as psp,
            tc.tile_pool(name="dramp", bufs=1, space="DRAM") as dramp,
        ):
            # ---- constants / inputs to SBUF ----
            xt = constp.tile([P, 8, TOK], dt.bfloat16, tag="xt")
            nc.sync.dma_start(xt, xT_d[:, :, :])
            b1_sb = constp.tile([P, 96], dt.float32, tag="b1")
            nc.sync.dma_start(b1_sb, b1_d[:, :])
            b2kq_sb = constp.tile([P, 16], dt.float32, tag="b2kq")
            nc.sync.dma_start(b2kq_sb, b2kq_d[:, :])
            b2v_sb = constp.tile([1, D], dt.bfloat16, tag="b2v")
            nc.sync.dma_start(b2v_sb, b2v_d[:, :])
            ones_sb = constp.tile([1, P], dt.bfloat16, tag="ones")
            nc.vector.memset(ones_sb, 1.0)

            # DRAM bounce buffers for the collectives
            k_dram = dramp.tile([D, TOK], dt.bfloat16, tag="k_dram")
            v_dram = dramp.tile([TOK, VROW], dt.bfloat16, tag="v_dram")
            kag_dram = dramp.tile([4 * D, TOK], dt.bfloat16, tag="kag_dram")
            vag_dram = dramp.tile([4 * TOK, VROW], dt.bfloat16, tag="vag_dram")

            groups = [[0, 1, 2, 3], [4, 5, 6, 7]]

            def mlp1(w1d, b1col0):
                """x -> silu(x w1 + b1), transposed layout h^T [HID, TOK] as 32 tiles."""
                hts = []
                for m in range(HID // P):
                    w1t = w1p.tile([P, 8, P], dt.bfloat16, tag="w1t")
                    nc.sync.dma_start(w1t, w1d[m, :, :, :])
                    ps = psp.tile([P, TOK], dt.float32, tag="ps")
                    for kk in range(8):
                        nc.tensor.matmul(
                            ps, lhsT=w1t[:, kk, :], rhs=xt[:, kk, :],
                            start=(kk == 0), stop=(kk == 7),
                        )
                    # silu(t) = t * sigmoid(t), t = ps + b1  (Silu is not in the
                    # interpreter; Sigmoid on ScalarE + fused mul on VectorE)
                    bcol = b1_sb[:, b1col0 + m : b1col0 + m + 1]
                    sg = kstp.tile([P, TOK], dt.float32, tag="sg")
                    nc.scalar.activation(sg, ps, AF.Sigmoid, bias=bcol)
                    ht = hp.tile([P, TOK], dt.bfloat16, tag="ht")
                    nc.vector.scalar_tensor_tensor(
                        ht, ps, bcol, sg,
                        op0=mybir.AluOpType.add, op1=mybir.AluOpType.mult,
                    )
                    hts.append(ht)
                return hts

            def mlp2_kq(w2d, b2col0, hts, dest):
                """h^T -> (silu_h w2 + b2)^T, D-major [D, TOK]; dest 'k' -> k_dram, 'q' -> SBUF."""
                q_tiles = []
                for md in range(D // P):
                    ps = psp.tile([P, TOK], dt.float32, tag="ps")
                    for kc in range(4):
                        w2t = w2p.tile([P, 8, P], dt.bfloat16, tag="w2t")
                        nc.sync.dma_start(w2t, w2d[md, :, 8 * kc : 8 * kc + 8, :])
                        for i in range(8):
                            kk = 8 * kc + i
                            nc.tensor.matmul(
                                ps, lhsT=w2t[:, i, :], rhs=hts[kk],
                                start=(kk == 0), stop=(kk == 31),
                            )
                    if dest == "k":
                        kst = kstp.tile([P, TOK], dt.bfloat16, tag="kst")
                        nc.scalar.activation(
                            kst, ps, AF.Identity, bias=b2kq_sb[:, b2col0 + md : b2col0 + md + 1]
                        )
                        nc.sync.dma_start(k_dram[P * md : P * (md + 1), :], kst)
                        if md % 2 == 1:
                            # AllGather this 256-row chunk of k^T right away
                            nc.gpsimd.collective_compute(
                                "AllGather", mybir.AluOpType.bypass,
                                replica_groups=groups,
                                ins=[k_dram[P * (md - 1) : P * (md + 1), :].opt()],
                                outs=[kag_dram[4 * P * (md - 1) : 4 * P * (md + 1), :].opt()],
                            )
                    else:
                        qt = qtp.tile([P, TOK], dt.bfloat16, tag="qt")
                        nc.scalar.activation(
                            qt, ps, AF.Identity, bias=b2kq_sb[:, b2col0 + md : b2col0 + md + 1]
                        )
                        q_tiles.append(qt)
                return q_tiles

            def mlp2_v(hts):
                """h^T -> v token-major [TOK, VROW] with ones cols, written to
                v_dram. Two half-passes (4 PSUM banks each) so the next MLP's
                accumulators are never starved; each 128-token tile is
                AllGathered as soon as it lands in DRAM."""
                for half in range(2):
                    vps = [
                        psp.tile([P, TOK], dt.float32, tag="ps", name=f"vps{half}{i}")
                        for i in range(4)
                    ]
                    for kk in range(HID // P):
                        w2vt = w2vp.tile([P, D], dt.bfloat16, tag="w2vt")
                        nc.sync.dma_start(w2vt, w2v_d[kk, :, :])
                        for mi in range(2):
                            mt = 2 * half + mi
                            for n2 in range(2):
                                nc.tensor.matmul(
                                    vps[mi * 2 + n2],
                                    lhsT=hts[kk][:, P * mt : P * (mt + 1)],
                                    rhs=w2vt[:, 512 * n2 : 512 * (n2 + 1)],
                                    start=(kk == 0), stop=False,
                                )
                    for mi in range(2):
                        mt = 2 * half + mi
                        for n2 in range(2):
                            nc.tensor.matmul(
                                vps[mi * 2 + n2], lhsT=ones_sb[0:1, 0:P],
                                rhs=b2v_sb[0:1, 512 * n2 : 512 * (n2 + 1)],
                                start=False, stop=True,
                            )
                        vst = vstp.tile([P, VROW], dt.bfloat16, tag="vst")
                        for n2 in range(2):
                            for h8 in range(8):
                                h = 8 * n2 + h8
                                nc.scalar.activation(
                                    vst[:, 65 * h : 65 * h + 64],
                                    vps[mi * 2 + n2][:, 64 * h8 : 64 * h8 + 64],
                                    AF.Copy,
                                )
                        nc.vector.memset(
                            vst.rearrange("p (h c) -> p h c", c=65)[:, :, 64:65], 1.0
                        )
                        nc.sync.dma_start(v_dram[P * mt : P * (mt + 1), :], vst)
                        nc.gpsimd.collective_compute(
                            "AllGather", mybir.AluOpType.bypass,
                            replica_groups=groups,
                            ins=[v_dram[P * mt : P * (mt + 1), :].opt()],
                            outs=[vag_dram[4 * P * mt : 4 * P * (mt + 1), :].opt()],
                        )

            # ---- MLPs: v, k, q (v first so its AllGather finishes earliest) ----
            hts = mlp1(w1_d["v"], 32)
            mlp2_v(hts)

            hts = mlp1(w1_d["k"], 0)
            mlp2_kq(w2kq_d["k"], 0, hts, "k")

            hts = mlp1(w1_d["q"], 64)
            q_tiles = mlp2_kq(w2kq_d["q"], 8, hts, "q")

            # ---- attention ----
            # Causal/validity masks (first needed here)
            masks_sb = constp.tile([P, NT, TOK], dt.bfloat16, tag="masks")
            nc.sync.dma_start(masks_sb, masks_d[:, :, :])
            # Load gathered v (whole batch) into SBUF: 16 tiles [128, 1040].
            # Chunked-AG layout: vag row-tile (4*lp + own).
            vag_sb = []
            for vt in range(16):
                vgt = vagp.tile([P, VROW], dt.bfloat16, tag="vgt")
                nc.sync.dma_start(vgt, vag_dram[P * vt : P * (vt + 1), :])
                vag_sb.append(vgt)

            for pair in range(H // 2):
                # k rows for heads (2*pair, 2*pair+1) from each of the 4 owners.
                # Chunked-AG layout: chunk (pair//2) holds [rank][256, 512].
                kag_sb = []
                for own in range(4):
                    kgt = kagp.tile([P, TOK], dt.bfloat16, tag="kgt")
                    rt = 8 * (pair // 2) + 2 * own + (pair % 2)
                    nc.sync.dma_start(kgt, kag_dram[P * rt : P * (rt + 1), :])
                    kag_sb.append(kgt)
                # Per head, two phases: (1) S^T + exp + mask for all key tiles
                # (kept in SBUF), (2) PV accumulation per query slot — one
                # PSUM accumulation group at a time (groups are bank-granular).
                for hh in range(2):
                    h = 2 * pair + hh
                    po = 64 * hh
                    exs = []
                    for j in range(NT):
                        own, blk = _owner(j)
                        c0 = P * blk  # first valid local query column
                        ps_s = psp.tile([P, TOK], dt.float32, tag="ps")
                        nc.tensor.matmul(
                            ps_s[:, c0:TOK],
                            lhsT=kag_sb[own][po : po + 64, P * blk : P * (blk + 1)],
                            rhs=q_tiles[h // 2][po : po + 64, c0:TOK],
                            start=True, stop=True,
                        )
                        ex = expp.tile([P, TOK], dt.bfloat16, tag="ex")
                        nc.scalar.activation(
                            ex[:, c0:TOK], ps_s[:, c0:TOK], AF.Exp, scale=0.125
                        )
                        nc.vector.tensor_mul(
                            ex[:, c0:TOK], ex[:, c0:TOK], masks_sb[:, j, c0:TOK]
                        )
                        exs.append(ex)
                    for p in range(4):
                        pvt = psp.tile([P, TOK], dt.float32, tag="ps", name=f"pvt{p}")
                        for j in range(4 * p + 4):
                            own, blk = _owner(j)
                            nc.tensor.matmul(
                                pvt[:, 0:65],
                                lhsT=exs[j][:, P * p : P * (p + 1)],
                                rhs=vag_sb[4 * blk + own][:, 65 * h : 65 * h + 65],
                                start=(j == 0), stop=(j == 4 * p + 3),
                            )
                        rec = outp.tile([P, 1], dt.float32, tag="rec")
                        nc.vector.reciprocal(rec, pvt[:, 64:65])
                        ot = outp.tile([P, 64], dt.float32, tag="ot")
                        nc.scalar.activation(ot, pvt[:, 0:64], AF.Copy, scale=rec)
                        nc.sync.dma_start(
                            o_d[P * p : P * (p + 1), 64 * h : 64 * h + 64], ot
                        )
    nc.compile()
    return nc


def _host_inputs(inputs):
    """Build the 8 per-core input maps from the full-problem inputs."""
    x = np.ascontiguousarray(inputs["x"]).astype(np.float32)

    def pack_w1(w1):
        return np.ascontiguousarray(
            w1.astype(BF16).reshape(8, P, HID // P, P).transpose(2, 1, 0, 3)
        )

    def pack_w2(w2):
        return np.ascontiguousarray(
            w2.astype(BF16).reshape(HID // P, P, D // P, P).transpose(2, 1, 0, 3)
        )

    shared = {
        "w1k": pack_w1(np.asarray(inputs["k_w1"])),
        "w1v": pack_w1(np.asarray(inputs["v_w1"])),
        "w1q": pack_w1(np.asarray(inputs["q_w1"])),
        "w2k": pack_w2(np.asarray(inputs["k_w2"])),
        "w2q": pack_w2(np.asarray(inputs["q_w2"])),
        "w2v": np.ascontiguousarray(
            np.asarray(inputs["v_w2"]).astype(BF16).reshape(HID // P, P, D)
        ),
        "b1": np.ascontiguousarray(
            np.concatenate(
                [np.asarray(inputs[m + "_b1"]).astype(np.float32).reshape(HID // P, P).T
                 for m in "kvq"], axis=1)
        ),
        "b2kq": np.ascontiguousarray(
            np.concatenate(
                [np.asarray(inputs[m + "_b2"]).astype(np.float32).reshape(D // P, P).T
                 for m in "kq"], axis=1)
        ),
        "b2v": np.ascontiguousarray(np.asarray(inputs["v_b2"]).astype(BF16).reshape(1, D)),
    }

    in_maps = []
    for c in range(NCORES):
        b, cc = divmod(c, 4)
        tiles = _tiles_for(cc)
        tok = np.concatenate([np.arange(P * t, P * (t + 1)) for t in tiles])
        xT = x[b].T[:, tok]  # [D, TOK]
        xT_packed = np.ascontiguousarray(
            xT.astype(BF16).reshape(8, P, TOK).transpose(1, 0, 2)
        )
        # mask[p_k, j, f_q] = 1 if key (128 j + p_k) <= query(local col f_q) else 0
        pk = np.arange(P)
        jj = np.arange(NT)
        fq = np.arange(TOK)
        qglob = np.array([P * tiles[f // P] + f % P for f in fq])  # [TOK]
        keyglob = P * jj[None, :, None] + pk[:, None, None]  # [P, NT, 1]
        mask = (keyglob <= qglob[None, None, :]).astype(BF16)
        in_maps.append({**shared, "xT": xT_packed, "masks": np.ascontiguousarray(mask)})
    return in_maps


LAST_RESULT = None


def kernel(**inputs):
    global LAST_RESULT
    key = "prog"
    if key not in _CACHE:
        _CACHE[key] = _build_program()
    nc = _CACHE[key]

    from concourse.bass_utils import run_bass_kernel_spmd

    in_maps = _host_inputs(inputs)
    res = run_bass_kernel_spmd(nc, in_maps, core_ids=list(range(NCORES)))
    LAST_RESULT = res

    full = np.zeros((B, S, D), np.float32)
    for c in range(NCORES):
        b, cc = divmod(c, 4)
        o_c = res.results[c]["o"]
        for p, t in enumerate(_tiles_for(cc)):
            full[b, P * t : P * (t + 1), :] = o_c[P * p : P * (p + 1), :]
    return full.astype(inputs["x"].dtype if hasattr(inputs["x"], "dtype") else np.float32)


# revision 22
# speedup vs baseline: 1.1626x; 1.0204x over previous
# Distributed Trainium2 kernel for the QKV-MLP + causal multi-head attention layer.
#
# Problem (all shapes fixed):
#   x [2, 2048, 1024];  q/k/v = MLP(x) with w1 [1024, 4096] (silu) and w2 [4096, 1024]
#   16 heads x 64 dims, causal softmax attention, output [2, 2048, 1024].
#
# Sharding (8 NeuronCores, single SPMD program; per-core behavior differs only
# through per-core input DATA, never through the instruction stream):
#   - Token/data parallel MLPs: each core computes q/k/v for 512 tokens of one
#     batch (cores 0-3 -> batch 0, 4-7 -> batch 1) with full (replicated) weights.
#   - Core quarter cc owns query tiles [cc, 7-cc, 8+cc, 15-cc] (128 tokens each),
#     which balances causal attention cost (sum of key-tiles = 34 for every cc).
#   - k^T (D-major) and v (token-major, with a ones-column per head for the
#     softmax denominator) are AllGathered within each 4-core batch group.
#   - Attention is query-local: per (head, key-tile) compute S^T = k_tile^T q,
#     exp on ScalarE, causal/validity masking via a host-precomputed per-core
#     mask, then PV accumulation in PSUM (ones-column yields the denominator).
#
# All matmul operands are bf16 (fp32 PSUM accumulation).

import os

import numpy as np
import ml_dtypes

B, S, D, HID, H, DH = 2, 2048, 1024, 4096, 16, 64
P = 128
NCORES = 8
TOK = 512           # tokens per core
NT = S // P         # 16 query/key tiles per batch
VROW = H * (DH + 1)  # 1040: v row with a ones column per head

BF16 = ml_dtypes.bfloat16

_CACHE = {}


def _tiles_for(cc):
    """Global query-tile indices (slot order) owned by quarter cc."""
    return [cc, 7 - cc, 8 + cc, 15 - cc]


def _owner(j):
    """Global tile j -> (owner quarter, slot index)."""
    blk = j // 4
    own = [j, 7 - j, j - 8, 15 - j][blk]
    return own, blk


def _build_program():
    import concourse.bass as bass
    import concourse.mybir as mybir
    import concourse.tile as tile
    from concourse import bacc

    dt = mybir.dt
    AF = mybir.ActivationFunctionType

    # Bacc (not raw Bass): its compile() splits multi-sem waits into event
    # semaphores, which TRN2 engine instructions require (max 1 wait each).
    nc = bacc.Bacc("TRN2", num_devices=NCORES)

    # ---- I/O ----
    xT_d = nc.dram_tensor("xT", [P, 8, TOK], dt.bfloat16, kind="ExternalInput")
    w1_d = {
        m: nc.dram_tensor(f"w1{m}", [HID // P, P, 8, P], dt.bfloat16, kind="ExternalInput")
        for m in "kvq"
    }
    w2kq_d = {
        m: nc.dram_tensor(f"w2{m}", [D // P, P, HID // P, P], dt.bfloat16, kind="ExternalInput")
        for m in "kq"
    }
    w2v_d = nc.dram_tensor("w2v", [HID // P, P, D], dt.bfloat16, kind="ExternalInput")
    b1_d = nc.dram_tensor("b1", [P, 96], dt.float32, kind="ExternalInput")
    b2kq_d = nc.dram_tensor("b2kq", [P, 16], dt.float32, kind="ExternalInput")
    b2v_d = nc.dram_tensor("b2v", [1, D], dt.bfloat16, kind="ExternalInput")
    masks_d = nc.dram_tensor("masks", [P, NT, P], dt.bfloat16, kind="ExternalInput")
    o_d = nc.dram_tensor("o", [TOK, D], dt.float32, kind="ExternalOutput")

    with tile.TileContext(nc) as tc:
        with (
            tc.tile_pool(name="constp", bufs=1) as constp,
            tc.tile_pool(name="w1p", bufs=6) as w1p,
            tc.tile_pool(name="w2p", bufs=4) as w2p,
            tc.tile_pool(name="w2vp", bufs=4) as w2vp,
            tc.tile_pool(name="hp", bufs=64) as hp,
            tc.tile_pool(name="qtp", bufs=8) as qtp,
            tc.tile_pool(name="kstp", bufs=3) as kstp,
            tc.tile_pool(name="vstp", bufs=2) as vstp,
            tc.tile_pool(name="kagp", bufs=8) as kagp,
            tc.tile_pool(name="vagp", bufs=16) as vagp,
            tc.tile_pool(name="expp", bufs=36) as expp,
            tc.tile_pool(name="outp", bufs=10) as outp,
            tc.tile_pool(name="psp", bufs=8, space="PSUM") as psp,
            tc.tile_pool(name="dramp", bufs=1, space="DRAM") as dramp,
        ):
            # ---- constants / inputs to SBUF ----
            xt = constp.tile([P, 8, TOK], dt.bfloat16, tag="xt")
            nc.sync.dma_start(xt, xT_d[:, :, :])
            b1_sb = constp.tile([P, 96], dt.float32, tag="b1")
            nc.sync.dma_start(b1_sb, b1_d[:, :])
            b2kq_sb = constp.tile([P, 16], dt.float32, tag="b2kq")
            nc.sync.dma_start(b2kq_sb, b2kq_d[:, :])
            b2v_sb = constp.tile([1, D], dt.bfloat16, tag="b2v")
            nc.sync.dma_start(b2v_sb, b2v_d[:, :])
            ones_sb = constp.tile([1, P], dt.bfloat16, tag="ones")
            nc.vector.memset(ones_sb, 1.0)

            # DRAM bounce buffers for the collectives
            k_dram = dramp.tile([D, TOK], dt.bfloat16, tag="k_dram")
            v_dram = dramp.tile([TOK, VROW], dt.bfloat16, tag="v_dram")
            kag_dram = dramp.tile([4 * D, TOK], dt.bfloat16, tag="kag_dram")
            vag_dram = dramp.tile([4 * TOK, VROW], dt.bfloat16, tag="vag_dram")

            groups = [[0, 1, 2, 3], [4, 5, 6, 7]]

            def mlp1(w1d, b1col0):
                """x -> silu(x w1 + b1), transposed layout h^T [HID, TOK] as 32 tiles."""
                hts = []
                for m in range(HID // P):
                    w1t = w1p.tile([P, 8, P], dt.bfloat16, tag="w1t")
                    nc.sync.dma_start(w1t, w1d[m, :, :, :])
                    ps = psp.tile([P, TOK], dt.float32, tag="ps")
                    for kk in range(8):
                        nc.tensor.matmul(
                            ps, lhsT=w1t[:, kk, :], rhs=xt[:, kk, :],
                            start=(kk == 0), stop=(kk == 7),
                        )
                    # silu(t) = t * sigmoid(t), t = ps + b1  (Silu is not in the
                    # interpreter; Sigmoid on ScalarE + fused mul on VectorE)
                    bcol = b1_sb[:, b1col0 + m : b1col0 + m + 1]
                    sg = kstp.tile([P, TOK], dt.float32, tag="sg")
                    nc.scalar.activation(sg, ps, AF.Sigmoid, bias=bcol)
                    ht = hp.tile([P, TOK], dt.bfloat16, tag="ht")
                    nc.vector.scalar_tensor_tensor(
                        ht, ps, bcol, sg,
                        op0=mybir.AluOpType.add, op1=mybir.AluOpType.mult,
                    )
                    hts.append(ht)
                return hts

            def mlp2_kq(w2d, b2col0, hts, dest):
                """h^T -> (silu_h w2 + b2)^T, D-major [D, TOK]; dest 'k' -> k_dram, 'q' -> SBUF."""
                q_tiles = []
                for md in range(D // P):
                    ps = psp.tile([P, TOK], dt.float32, tag="ps")
                    for kc in range(4):
                        w2t = w2p.tile([P, 8, P], dt.bfloat16, tag="w2t")
                        nc.sync.dma_start(w2t, w2d[md, :, 8 * kc : 8 * kc + 8, :])
                        for i in range(8):
                            kk = 8 * kc + i
                            nc.tensor.matmul(
                                ps, lhsT=w2t[:, i, :], rhs=hts[kk],
                                start=(kk == 0), stop=(kk == 31),
                            )
                    if dest == "k":
                        kst = kstp.tile([P, TOK], dt.bfloat16, tag="kst")
                        nc.scalar.activation(
                            kst, ps, AF.Identity, bias=b2kq_sb[:, b2col0 + md : b2col0 + md + 1]
                        )
                        nc.sync.dma_start(k_dram[P * md : P * (md + 1), :], kst)
                        if md % 2 == 1:
                            # AllGather this 256-row chunk of k^T right away
                            nc.gpsimd.collective_compute(
                                "AllGather", mybir.AluOpType.bypass,
                                replica_groups=groups,
                                ins=[k_dram[P * (md - 1) : P * (md + 1), :].opt()],
                                outs=[kag_dram[4 * P * (md - 1) : 4 * P * (md + 1), :].opt()],
                            )
                    else:
                        qt = qtp.tile([P, TOK], dt.bfloat16, tag="qt")
                        nc.scalar.activation(
                            qt, ps, AF.Identity, bias=b2kq_sb[:, b2col0 + md : b2col0 + md + 1]
                        )
                        q_tiles.append(qt)
                return q_tiles

            def mlp2_v(hts):
                """h^T -> v token-major [TOK, VROW] with ones cols, written to
                v_dram. Two half-passes (4 PSUM banks each) so the next MLP's
                accumulators are never starved; each 128-token tile is
                AllGathered as soon as it lands in DRAM."""
                for half in range(2):
                    vps = [
                        psp.tile([P, TOK], dt.float32, tag="ps", name=f"vps{half}{i}")
                        for i in range(4)
                    ]
                    for kk in range(HID // P):
                        w2vt = w2vp.tile([P, D], dt.bfloat16, tag="w2vt")
                        nc.sync.dma_start(w2vt, w2v_d[kk, :, :])
                        for mi in range(2):
                            mt = 2 * half + mi
                            for n2 in range(2):
                                nc.tensor.matmul(
                                    vps[mi * 2 + n2],
                                    lhsT=hts[kk][:, P * mt : P * (mt + 1)],
                                    rhs=w2vt[:, 512 * n2 : 512 * (n2 + 1)],
                                    start=(kk == 0), stop=False,
                                )
                    for mi in range(2):
                        mt = 2 * half + mi
                        for n2 in range(2):
                            nc.tensor.matmul(
                                vps[mi * 2 + n2], lhsT=ones_sb[0:1, 0:P],
                                rhs=b2v_sb[0:1, 512 * n2 : 512 * (n2 + 1)],
                                start=False, stop=True,
                            )
                        vst = vstp.tile([P, VROW], dt.bfloat16, tag="vst")
                        for n2 in range(2):
                            for h8 in range(8):
                                h = 8 * n2 + h8
                                nc.scalar.activation(
                                    vst[:, 65 * h : 65 * h + 64],
                                    vps[mi * 2 + n2][:, 64 * h8 : 64 * h8 + 64],
                                    AF.Copy,
                                )
                        nc.vector.memset(
                            vst.rearrange("p (h c) -> p h c", c=65)[:, :, 64:65], 1.0
                        )
                        nc.sync.dma_start(v_dram[P * mt : P * (mt + 1), :], vst)
                        nc.gpsimd.collective_compute(
                            "AllGather", mybir.AluOpType.bypass,
                            replica_groups=groups,
                            ins=[v_dram[P * mt : P * (mt + 1), :].opt()],
                            outs=[vag_dram[4 * P * mt : 4 * P * (mt + 1), :].opt()],
                        )

            # ---- MLPs: v, k, q (v first so its AllGather finishes earliest) ----
            hts = mlp1(w1_d["v"], 32)
            mlp2_v(hts)

            hts = mlp1(w1_d["k"], 0)
            mlp2_kq(w2kq_d["k"], 0, hts, "k")

            hts = mlp1(w1_d["q"], 64)
            q_tiles = mlp2_kq(w2kq_d["q"], 8, hts, "q")

            # ---- attention ----
            # Causal/validity masks (first needed here)
            masks_sb = constp.tile([P, NT, P], dt.bfloat16, tag="masks")
            nc.sync.dma_start(masks_sb, masks_d[:, :, :])
            # Load gathered v (whole batch) into SBUF: 16 tiles [128, 1040].
            # Chunked-AG layout: vag row-tile (4*lp + own).
            vag_sb = []
            for vt in range(16):
                vgt = vagp.tile([P, VROW], dt.bfloat16, tag="vgt")
                nc.sync.dma_start(vgt, vag_dram[P * vt : P * (vt + 1), :])
                vag_sb.append(vgt)

            for pair in range(H // 2):
                # k rows for heads (2*pair, 2*pair+1) from each of the 4 owners.
                # Chunked-AG layout: chunk (pair//2) holds [rank][256, 512].
                kag_sb = []
                for own in range(4):
                    kgt = kagp.tile([P, TOK], dt.bfloat16, tag="kgt")
                    rt = 8 * (pair // 2) + 2 * own + (pair % 2)
                    nc.sync.dma_start(kgt, kag_dram[P * rt : P * (rt + 1), :])
                    kag_sb.append(kgt)
                # Per head, two phases: (1) S^T + exp + mask for all key tiles
                # (kept in SBUF), (2) PV accumulation per query slot — one
                # PSUM accumulation group at a time (groups are bank-granular).
                for hh in range(2):
                    h = 2 * pair + hh
                    po = 64 * hh
                    exs = []
                    for j in range(NT):
                        own, blk = _owner(j)
                        c0 = P * blk  # first valid local query column
                        ps_s = psp.tile([P, TOK], dt.float32, tag="ps")
                        nc.tensor.matmul(
                            ps_s[:, c0:TOK],
                            lhsT=kag_sb[own][po : po + 64, P * blk : P * (blk + 1)],
                            rhs=q_tiles[h // 2][po : po + 64, c0:TOK],
                            start=True, stop=True,
                        )
                        ex = expp.tile([P, TOK], dt.bfloat16, tag="ex")
                        nc.scalar.activation(
                            ex[:, c0:TOK], ps_s[:, c0:TOK], AF.Exp, scale=0.125
                        )
                        # only the diagonal 128-col slot ever needs masking
                        nc.vector.tensor_mul(
                            ex[:, c0 : c0 + P], ex[:, c0 : c0 + P], masks_sb[:, j, :]
                        )
                        exs.append(ex)
                    for p in range(4):
                        pvt = psp.tile([P, TOK], dt.float32, tag="ps", name=f"pvt{p}")
                        for j in range(4 * p + 4):
                            own, blk = _owner(j)
                            nc.tensor.matmul(
                                pvt[:, 0:65],
                                lhsT=exs[j][:, P * p : P * (p + 1)],
                                rhs=vag_sb[4 * blk + own][:, 65 * h : 65 * h + 65],
                                start=(j == 0), stop=(j == 4 * p + 3),
                            )
                        rec = outp.tile([P, 1], dt.float32, tag="rec")
                        nc.vector.reciprocal(rec, pvt[:, 64:65])
                        ot = outp.tile([P, 64], dt.float32, tag="ot")
                        nc.scalar.activation(ot, pvt[:, 0:64], AF.Copy, scale=rec)
                        nc.sync.dma_start(
                            o_d[P * p : P * (p + 1), 64 * h : 64 * h + 64], ot
                        )
    nc.compile()
    return nc


def _host_inputs(inputs):
    """Build the 8 per-core input maps from the full-problem inputs."""
    x = np.ascontiguousarray(inputs["x"]).astype(np.float32)

    def pack_w1(w1):
        return np.ascontiguousarray(
            w1.astype(BF16).reshape(8, P, HID // P, P).transpose(2, 1, 0, 3)
        )

    def pack_w2(w2):
        return np.ascontiguousarray(
            w2.astype(BF16).reshape(HID // P, P, D // P, P).transpose(2, 1, 0, 3)
        )

    shared = {
        "w1k": pack_w1(np.asarray(inputs["k_w1"])),
        "w1v": pack_w1(np.asarray(inputs["v_w1"])),
        "w1q": pack_w1(np.asarray(inputs["q_w1"])),
        "w2k": pack_w2(np.asarray(inputs["k_w2"])),
        "w2q": pack_w2(np.asarray(inputs["q_w2"])),
        "w2v": np.ascontiguousarray(
            np.asarray(inputs["v_w2"]).astype(BF16).reshape(HID // P, P, D)
        ),
        "b1": np.ascontiguousarray(
            np.concatenate(
                [np.asarray(inputs[m + "_b1"]).astype(np.float32).reshape(HID // P, P).T
                 for m in "kvq"], axis=1)
        ),
        "b2kq": np.ascontiguousarray(
            np.concatenate(
                [np.asarray(inputs[m + "_b2"]).astype(np.float32).reshape(D // P, P).T
                 for m in "kq"], axis=1)
        ),
        "b2v": np.ascontiguousarray(np.asarray(inputs["v_b2"]).astype(BF16).reshape(1, D)),
    }

    in_maps = []
    for c in range(NCORES):
        b, cc = divmod(c, 4)
        tiles = _tiles_for(cc)
        tok = np.concatenate([np.arange(P * t, P * (t + 1)) for t in tiles])
        xT = x[b].T[:, tok]  # [D, TOK]
        xT_packed = np.ascontiguousarray(
            xT.astype(BF16).reshape(8, P, TOK).transpose(1, 0, 2)
        )
        # mask[p_k, j, f_q] = 1 if key (128 j + p_k) <= query(local col f_q) else 0
        pk = np.arange(P)
        jj = np.arange(NT)
        fq = np.arange(P)
        # mask for the diagonal slot only: local query col = 128*blk(j) + f
        qglob = np.array(
            [[P * tiles[j // 4] + f for f in fq] for j in jj]
        )  # [NT, P]
        keyglob = P * jj[None, :, None] + pk[:, None, None]  # [P, NT, 1]
        mask = (keyglob <= qglob[None, :, :]).astype(BF16)
        in_maps.append({**shared, "xT": xT_packed, "masks": np.ascontiguousarray(mask)})
    return in_maps


LAST_RESULT = None


def kernel(**inputs):
    global LAST_RESULT
    key = "prog"
    if key not in _CACHE:
        _CACHE[key] = _build_program()
    nc = _CACHE[key]

    from concourse.bass_utils import run_bass_kernel_spmd

    in_maps = _host_inputs(inputs)
    res = run_bass_kernel_spmd(nc, in_maps, core_ids=list(range(NCORES)))
    LAST_RESULT = res

    full = np.zeros((B, S, D), np.float32)
    for c in range(NCORES):
        b, cc = divmod(c, 4)
        o_c = res.results[c]["o"]
        for p, t in enumerate(_tiles_for(cc)):
            full[b, P * t : P * (t + 1), :] = o_c[P * p : P * (p + 1), :]
    return full.astype(inputs["x"].dtype if hasattr(inputs["x"], "dtype") else np.float32)
